# revision 36
# baseline (speedup 1.0000x reference)
"""Trainium2 Bass kernel for nn_CODEXReconstruction (moe_routing).

Data-parallel over the batch across 8 NeuronCores; all weights replicated.
Per-core pipeline (batch shard B=1024, activations stored transposed
[features, batch] so every layer's weight tensor is used directly as the
matmul stationary operand and no on-device transposes are needed):

    enc1:  h1  = relu(W1.T @ xT + b1)      [512, 1024]
    enc2:  emb = relu(W2.T @ h1 + b2)      [256, 1024]
    experts (t = 0..19):
           ps_t = TW[t].T @ emb            (2 k-tiles, f32 PSUM)
           ps_t += (gate[t,b]-1)*1e30      (4 concurrent K=1 row-tiled
                  matmuls, one per PSUM quadrant, via tile_position —
                  ~4x cheaper than a full-width gate matmul)
           lat  += relu(ps_t + Tb[t])      (relu on ACT or DVE, add on DVE
                  in bf16; ping-pong accumulators)
    dec1/dec2: relu matmuls                [512, 1024]
    dec3:  rec = W3.T @ d2 + b3            [10000, 1024]
           rows <5000: copy out; rows >=5000: softplus + 0.001 via exp/ln

All matmuls run bf16 (f32 PSUM); outputs are written fp16. DMA streams are
packed 4 k-tiles per step (4-8KB contiguous DRAM rows -> bigger DMA packets;
the 16 shared DMA engines are packet-rate-bound and were the enc1
bottleneck). Non-critical constant loads (dec weights, dec biases) are
deferred into the expert phase to keep enc1's x/w1 stream fed. The HAM
warm-up covers just the initial DMA latency. dec3 interleaves vars (exp/ln
on ACT) two pairs ahead of means (bias-add on DVE) so the kernel tail ends
on cheap means tiles.
"""

import numpy as np
import ml_dtypes

import bass_rust
import concourse.bass as bass
import concourse.mybir as mybir
import concourse.tile as tile
from concourse.bass_utils import run_bass_kernel_spmd
from concourse.tile import ScopedClock

# ---------------------------------------------------------------------------
# Problem constants (hardcoded per contract)
# ---------------------------------------------------------------------------
IN_F = 5000
IN_FP = 5120                  # zero-padded K so k-tiles are uniform 128
N0, N1, N2 = 512, 512, 256
T = 20
BATCH = 8192
N_CORES = 8
B = BATCH // N_CORES          # 1024 per core
NB = B // 512                 # moving-dim chunks of 512
KP4 = IN_FP // 512            # 10 packed x/w1 stream steps (4 k-tiles each)
MT_HALF = 40                  # 5000 out-features -> 40 m-tiles (last 8 valid)

F32 = mybir.dt.float32
F16 = mybir.dt.float16
BF16 = mybir.dt.bfloat16
RELU = mybir.ActivationFunctionType.Relu
IDENT = mybir.ActivationFunctionType.Identity
EXP = mybir.ActivationFunctionType.Exp
LN = mybir.ActivationFunctionType.Ln
ADD = mybir.AluOpType.add
MULT = mybir.AluOpType.mult
MAX = mybir.AluOpType.max
# softplus(x)+0.001 = ln(C + C*e^x) with C = e^0.001 (this walrus build has no
# Softplus act table; exp/ln/relu/identity all live in one table set)
SP_C = 1.0010005001667084

# ---------------------------------------------------------------------------
# Workaround: this walrus build rejects >1 sync wait per instruction.
# Split extra waits onto injected same-engine NoOps (engine streams are
# in-order, so a preceding same-engine wait is equivalent), and chunk the
# Tile tail-drain's waits across chained drain instructions.
# ---------------------------------------------------------------------------
_uid = [0]


def _nop_with_wait(engine, wait):
    _uid[0] += 1
    nop = mybir.InstNoOp(name=f"WSPLIT-{_uid[0]}", ins=[], outs=[])
    nop.engine = engine
    nop.sync_info = bass_rust.SyncInfo(on_wait=[wait], on_update=[])
    return nop


def split_sync_waits(nc):
    for f in nc.m.functions:
        for bb in f.blocks:
            old = bb.instructions
            if not any(
                i.sync_info and i.sync_info.on_wait and len(i.sync_info.on_wait) > 1
                for i in old
            ):
                continue
            new = []
            for inst in old:
                si = inst.sync_info
                if si is not None and si.on_wait and len(si.on_wait) > 1:
                    waits = list(si.on_wait)
                    for w in waits[:-1]:
                        new.append(_nop_with_wait(inst.engine, w))
                    si.on_wait = [waits[-1]]
                new.append(inst)
            bb.instructions = new


def _patched_drain_and_barrier(self, tick_clock, wait_clock):
    nc = self.nc
    drain_inst = nc.sync.drain()
    wait_clock.add_sem_waits(
        drain_inst.ins, ScopedClock({None: tick_clock.global_clock})
    )
    waits = list(drain_inst.ins.sync_info.on_wait or [])
    if len(waits) > 1:
        drain_inst.ins.sync_info.on_wait = waits[:1]
        for i in range(1, len(waits)):
            extra = nc.sync.drain()
            if extra.ins.sync_info is None:
                extra.ins.sync_info = bass_rust.SyncInfo(
                    on_wait=[waits[i]], on_update=[]
                )
            else:
                extra.ins.sync_info.on_wait = [waits[i]]

    nc.all_engine_barrier()
    assert self.sems is not None
    popped = nc._tile_sem_poison_stack.pop()
    assert popped is self._sem_poison
    nc.clear_and_free_semaphores(list(self.sems.allocated().values()))
    nc.all_engine_barrier()


tile.TileContext._drain_and_barrier = _patched_drain_and_barrier


# ---------------------------------------------------------------------------
# Bass module (one NeuronCore's program; SPMD across 8 cores)
# ---------------------------------------------------------------------------
def build_bass():
    nc = bass.Bass()

    # packed streams: per step j, x/w1 hold k-tiles 4j..4j+3 side by side
    xp = nc.dram_tensor("xp", [KP4, 128, 4 * B], BF16, kind="ExternalInput")
    w1p = nc.dram_tensor("w1p", [KP4, 128, 4 * N0], BF16, kind="ExternalInput")
    w2 = nc.dram_tensor("w2", [N0, N2], BF16, kind="ExternalInput")
    twp = nc.dram_tensor("twp", [T, 128, 2 * N2], BF16, kind="ExternalInput")
    # gate rows replicated into the 4 PE row-groups: gmrep[32i+t, b] =
    # (gate[t,b]-1)*1e30 for i in 0..3
    gmrep = nc.dram_tensor("gmrep", [128, B], BF16, kind="ExternalInput")
    # one-hot selector: ohsel[32i+t, t*128+c] = 1 — stationary [20,128]
    # slices at 32-aligned partitions for the row-tiled gate matmuls
    ohsel = nc.dram_tensor("ohsel", [128, T * 128], BF16, kind="ExternalInput")
    dw1 = nc.dram_tensor("dw1", [N2, N1], BF16, kind="ExternalInput")
    dw2 = nc.dram_tensor("dw2", [N1, N0], BF16, kind="ExternalInput")
    # mi-pairs packed: w3*[j, p, mi2*512 + k*128 + c] = W3[k*128+p, (2j+mi2)*128+c]
    w3m = nc.dram_tensor("w3m", [MT_HALF // 2, 128, 1024], BF16, kind="ExternalInput")
    w3v = nc.dram_tensor("w3v", [MT_HALF // 2, 128, 1024], BF16, kind="ExternalInput")
    # bias columns: [128, n_tiles], col j = bias[j*128 : (j+1)*128]
    b1c = nc.dram_tensor("b1c", [128, 4], F32, kind="ExternalInput")
    b2c = nc.dram_tensor("b2c", [128, 2], F32, kind="ExternalInput")
    tbc = nc.dram_tensor("tbc", [128, T * 2], F32, kind="ExternalInput")
    db1c = nc.dram_tensor("db1c", [128, 4], F32, kind="ExternalInput")
    db2c = nc.dram_tensor("db2c", [128, 4], F32, kind="ExternalInput")
    b3mc = nc.dram_tensor("b3mc", [128, MT_HALF], F32, kind="ExternalInput")
    b3vc = nc.dram_tensor("b3vc", [128, MT_HALF], F32, kind="ExternalInput")

    yt = nc.dram_tensor("yt", [2 * IN_F, B], F16, kind="ExternalOutput")

    with tile.TileContext(nc) as tc:
        with (
            tc.tile_pool(name="const", bufs=1) as const,
            tc.tile_pool(name="acts", bufs=8) as acts,
            tc.tile_pool(name="acc", bufs=4) as accp,
            tc.tile_pool(name="xs", bufs=7) as xs,
            tc.tile_pool(name="ws", bufs=5) as wsp,
            tc.tile_pool(name="tws", bufs=3) as tws,
            tc.tile_pool(name="w3s", bufs=4) as w3s,
            tc.tile_pool(name="outs", bufs=3) as outs,
            tc.tile_pool(name="rp", bufs=3) as rp,
            tc.tile_pool(name="ps", bufs=4, space="PSUM") as psp,
        ):
            # ------- HAM warm-up: dummy matmuls bridge the initial DMA
            # latency so the PE activity window never goes idle before enc1
            # A long HAM warm-up is counterproductive: isolated LDW+MM pairs
            # at the cold clock cost ~450ns each and delay enc1. Just start
            # PE activity immediately so the HAM busy-window opens.
            warm = const.tile([128, 128], BF16, name="warm")
            nc.vector.memset(warm[:], 0.0)
            wps = psp.tile([128, 128], F32, name="wps", tag="ps")
            NWARM = 4
            for i in range(NWARM):
                nc.tensor.matmul(
                    wps[:], warm[:], warm[:], start=(i == 0), stop=(i == NWARM - 1)
                )

            # ------- enc1 stream layout: per step, x halves ride sync+scalar
            # and w1 rides gpsimd — 512KB per queue per step, so round-robin
            # DMA-engine service delivers in exactly consumption order. Each
            # queue's in-order packet stream is its own throttle; DMA issue
            # instructions cost ~600ns each, so steps use few, large DMAs
            # (j=0 is quartered only so the first matmul fires ~1.6us in).
            pass

            # persistent constants: DECLARED here, but their DMAs ride the
            # scalar queue and are EMITTED after the w1 stream loads below —
            # the wsp pool rotation then delays their issue to ~40us, keeping
            # the DMA engines free for the enc1 x/w1 stream.
            oh_sb = const.tile([128, T * 128], BF16, name="oh_sb")
            w2_sb = [
                const.tile([128, N2], BF16, name=f"w2_{k}") for k in range(4)
            ]
            gm_sb = const.tile([128, B], BF16, name="gm_sb")

            def load_bias(name, src, cols):
                t_ = const.tile([128, cols], F32, name=name)
                nc.scalar.dma_start(out=t_[:], in_=src[:])
                return t_

            def mk_psum(tag_name):
                # [128, 1024] = 2 PSUM banks; matmuls fill 512-wide halves
                return psp.tile([128, B], F32, name=tag_name, tag="ps")

            # ------- enc1 (bf16): [5120,1024] -> [512,1024]
            h1 = [
                acts.tile([128, B], BF16, name=f"h1_{m}", tag="a1024")
                for m in range(4)
            ]
            ps_h1 = [mk_psum(f"psh1_{m}") for m in range(4)]
            for j in range(KP4):
                xk = xs.tile([128, 4 * B], BF16, name=f"x_{j}", tag="x")
                w1k = wsp.tile([128, 4 * N0], BF16, name=f"w1_{j}", tag="w")
                if j == 0:
                    # each queue's FIRST transfer is exactly what the first
                    # matmuls consume first: w1 u0-half and x u0-half lead on
                    # scalar/sync (in run9 w1-j0 sat behind gpsimd's slow
                    # queue and the first matmul waited until ~15us)
                    nc.scalar.dma_start(out=w1k[:, :N0], in_=w1p[0, :, :N0])
                    nc.sync.dma_start(out=xk[:, :512], in_=xp[0, :, :512])
                    nc.sync.dma_start(out=xk[:, 512:B], in_=xp[0, :, 512:B])
                    nc.gpsimd.dma_start(
                        out=xk[:, 2 * B:3 * B], in_=xp[0, :, 2 * B:3 * B]
                    )
                    nc.scalar.dma_start(out=w1k[:, N0:2 * N0], in_=w1p[0, :, N0:2 * N0])
                    nc.sync.dma_start(out=xk[:, B:2 * B], in_=xp[0, :, B:2 * B])
                    nc.gpsimd.dma_start(out=xk[:, 3 * B:], in_=xp[0, :, 3 * B:])
                    nc.scalar.dma_start(out=w1k[:, 2 * N0:], in_=w1p[0, :, 2 * N0:])
                else:
                    nc.gpsimd.dma_start(out=w1k[:], in_=w1p[j])
                    nc.sync.dma_start(out=xk[:, :2 * B], in_=xp[j, :, :2 * B])
                    nc.scalar.dma_start(out=xk[:, 2 * B:], in_=xp[j, :, 2 * B:])
                for u in range(4):
                    for m in range(4):
                        for n in range(NB):
                            nc.tensor.matmul(
                                ps_h1[m][:, n * 512:(n + 1) * 512],
                                w1k[:, u * N0 + m * 128: u * N0 + (m + 1) * 128],
                                xk[:, u * B + n * 512: u * B + (n + 1) * 512],
                                start=(j == 0 and u == 0),
                                stop=(j == KP4 - 1 and u == 3),
                            )
            # deferred const DMAs: queued on the scalar engine behind the
            # pool-throttled w1 loads above, so they issue ~40us in and never
            # contend with the enc1 stream. In-order queues + tile deps keep
            # them correct; all land well before their consumers.
            for k in range(4):
                nc.scalar.dma_start(out=w2_sb[k][:], in_=w2[k * 128:(k + 1) * 128, :])
            nc.scalar.dma_start(out=gm_sb[:], in_=gmrep[:])
            nc.scalar.dma_start(out=oh_sb[:], in_=ohsel[:])
            b1_sb = load_bias("b1_sb", b1c, 4)
            b2_sb = load_bias("b2_sb", b2c, 2)
            tb_sb = load_bias("tb_sb", tbc, T * 2)
            tw_pre = []
            for t in range(2):
                twk = tws.tile([128, 2 * N2], BF16, name=f"tw_{t}", tag="tw")
                nc.scalar.dma_start(out=twk[:], in_=twp[t])
                tw_pre.append(twk)

            # chunked relu epilogue so enc2 matmuls start after the first chunk
            for m in range(4):
                for n in range(NB):
                    sl = slice(n * 512, (n + 1) * 512)
                    nc.scalar.activation(
                        h1[m][:, sl], ps_h1[m][:, sl], RELU, bias=b1_sb[:, m:m + 1]
                    )

            # ------- enc2: [512,1024] -> [256,1024]
            emb = [
                acts.tile([128, B], BF16, name=f"emb_{m}", tag="a1024")
                for m in range(2)
            ]
            ps_e = [mk_psum(f"pse_{m}") for m in range(2)]
            for k in range(4):
                for m in range(2):
                    for n in range(NB):
                        nc.tensor.matmul(
                            ps_e[m][:, n * 512:(n + 1) * 512],
                            w2_sb[k][:, m * 128:(m + 1) * 128],
                            h1[k][:, n * 512:(n + 1) * 512],
                            start=(k == 0),
                            stop=(k == 3),
                        )
            for n in range(NB):
                for m in range(2):
                    sl = slice(n * 512, (n + 1) * 512)
                    nc.scalar.activation(
                        emb[m][:, sl], ps_e[m][:, sl], RELU, bias=b2_sb[:, m:m + 1]
                    )

            # ------- experts + gated accumulation
            # bf16 ping-pong accumulators: TT adds ride the DVE 2x path
            lat = [
                [
                    accp.tile([128, B], BF16, name=f"lat_{f}_{p}", tag="lacc")
                    for p in range(2)
                ]
                for f in range(2)
            ]
            dw1_sb = []
            dw2_sb = []
            db1_sb = db2_sb = b3m_sb = b3v_sb = None
            for t in range(T):
                if t < 2:
                    twk = tw_pre[t]
                else:
                    twk = tws.tile([128, 2 * N2], BF16, name=f"tw_{t}", tag="tw")
                    nc.gpsimd.dma_start(out=twk[:], in_=twp[t])
                if t == 3:
                    # deferred constant loads: needed only from dec1 (~120us)
                    # on; issuing them here keeps enc1's DMA stream fed
                    for k in range(2):
                        t_ = const.tile([128, N1], BF16, name=f"dw1_{k}")
                        nc.gpsimd.dma_start(
                            out=t_[:], in_=dw1[k * 128:(k + 1) * 128, :]
                        )
                        dw1_sb.append(t_)
                    for k in range(4):
                        t_ = const.tile([128, N0], BF16, name=f"dw2_{k}")
                        nc.gpsimd.dma_start(
                            out=t_[:], in_=dw2[k * 128:(k + 1) * 128, :]
                        )
                        dw2_sb.append(t_)
                    db1_sb = load_bias("db1_sb", db1c, 4)
                    db2_sb = load_bias("db2_sb", db2c, 4)
                    b3m_sb = load_bias("b3m_sb", b3mc, MT_HALF)
                    b3v_sb = load_bias("b3v_sb", b3vc, MT_HALF)
                ps = [mk_psum(f"pst_{t}_{f}") for f in range(2)]
                for f in range(2):
                    for k in range(2):
                        for n in range(NB):
                            nc.tensor.matmul(
                                ps[f][:, n * 512:(n + 1) * 512],
                                twk[:, k * N2 + f * 128: k * N2 + (f + 1) * 128],
                                emb[k][:, n * 512:(n + 1) * 512],
                                start=(k == 0),
                                stop=False,
                            )
                # gate offsets: 4 concurrent K=1 matmuls, one per PSUM
                # quadrant, in the 4 PE row groups (tile_position) — the
                # whole gate costs ~1 matmul's streaming time
                for f in range(2):
                    for n in range(NB):
                        i = 2 * f + n
                        p0 = 32 * i
                        nc.tensor.matmul(
                            ps[f][:, n * 512:(n + 1) * 512],
                            oh_sb[p0:p0 + T, t * 128:(t + 1) * 128],
                            gm_sb[p0:p0 + T, n * 512:(n + 1) * 512],
                            start=False,
                            stop=True,
                            tile_position=(p0, 0),
                        )
                if t == T - 1:
                    # last expert: f1 relu on ACT in parallel with a fully
                    # chunked DVE relu+add chain for f0, so dec1's k=0
                    # matmuls start ~1.1us after the last gate matmul
                    r1 = rp.tile([128, B], BF16, name=f"r_{t}_1", tag="r")
                    nc.scalar.activation(
                        r1[:], ps[1][:], RELU, bias=tb_sb[:, t * 2 + 1:t * 2 + 2]
                    )
                    r0 = rp.tile([128, B], BF16, name=f"r_{t}_0", tag="r")
                    for n in range(NB):
                        sl = slice(n * 512, (n + 1) * 512)
                        nc.vector.tensor_scalar(
                            r0[:, sl], ps[0][:, sl],
                            tb_sb[:, t * 2:t * 2 + 1], 0.0, op0=ADD, op1=MAX,
                        )
                        nc.vector.tensor_add(
                            lat[0][t % 2][:, sl], lat[0][(t - 1) % 2][:, sl],
                            r0[:, sl],
                        )
                    for n in range(NB):
                        sl = slice(n * 512, (n + 1) * 512)
                        nc.vector.tensor_add(
                            lat[1][t % 2][:, sl], lat[1][(t - 1) % 2][:, sl],
                            r1[:, sl],
                        )
                    continue
                for f in range(2):
                    bias_ap = tb_sb[:, t * 2 + f:t * 2 + f + 1]
                    if t == 0:
                        nc.scalar.activation(lat[f][0][:], ps[f][:], RELU, bias=bias_ap)
                        continue
                    r = rp.tile([128, B], BF16, name=f"r_{t}_{f}", tag="r")
                    if f == 1 and t % 2 == 1:
                        # spill some relus to the DVE so ACT isn't the
                        # expert-phase critical engine
                        nc.vector.tensor_scalar(
                            r[:], ps[f][:], bias_ap, 0.0, op0=ADD, op1=MAX
                        )
                    else:
                        nc.scalar.activation(r[:], ps[f][:], RELU, bias=bias_ap)
                    nc.vector.tensor_add(
                        lat[f][t % 2][:], lat[f][(t - 1) % 2][:], r[:]
                    )
            latf = [lat[f][(T - 1) % 2] for f in range(2)]

            # ------- dec1: [256,1024] -> [512,1024]
            d1 = [
                acts.tile([128, B], BF16, name=f"d1_{m}", tag="a1024")
                for m in range(4)
            ]
            ps_d1 = [mk_psum(f"psd1_{m}") for m in range(4)]
            for k in range(2):
                for m in range(4):
                    for n in range(NB):
                        nc.tensor.matmul(
                            ps_d1[m][:, n * 512:(n + 1) * 512],
                            dw1_sb[k][:, m * 128:(m + 1) * 128],
                            latf[k][:, n * 512:(n + 1) * 512],
                            start=(k == 0),
                            stop=(k == 1),
                        )
            for m in range(4):
                for n in range(NB):
                    sl = slice(n * 512, (n + 1) * 512)
                    nc.scalar.activation(
                        d1[m][:, sl], ps_d1[m][:, sl], RELU, bias=db1_sb[:, m:m + 1]
                    )

            # ------- dec2: [512,1024] -> [512,1024]
            d2 = [
                acts.tile([128, B], BF16, name=f"d2_{m}", tag="a1024")
                for m in range(4)
            ]
            ps_d2 = [mk_psum(f"psd2_{m}") for m in range(4)]
            for k in range(4):
                for m in range(4):
                    for n in range(NB):
                        nc.tensor.matmul(
                            ps_d2[m][:, n * 512:(n + 1) * 512],
                            dw2_sb[k][:, m * 128:(m + 1) * 128],
                            d1[k][:, n * 512:(n + 1) * 512],
                            start=(k == 0),
                            stop=(k == 3),
                        )
            for m in range(4):
                for n in range(NB):
                    sl = slice(n * 512, (n + 1) * 512)
                    nc.scalar.activation(
                        d2[m][:, sl], ps_d2[m][:, sl], RELU, bias=db2_sb[:, m:m + 1]
                    )

            # ------- dec3 (bf16) + output heads. vars pairs (2 ACT
            # transcendental passes each) are scheduled two slots ahead of
            # means pairs so the ACT tail drains under means PE work and the
            # kernel ends on cheap means tiles.
            def dec3_pair(wsrc, bias_sb, out_row0, softplus, j, split_store):
                w3k = w3s.tile(
                    [128, 1024], BF16, name=f"w3_{out_row0}_{j}", tag="w3"
                )
                nc.gpsimd.dma_start(out=w3k[:], in_=wsrc[j])
                o = outs.tile([128, 2 * B], F16, name=f"o_{out_row0}_{j}", tag="o")
                for mi2 in range(2):
                    mi = 2 * j + mi2
                    mw = 128 if mi < MT_HALF - 1 else (IN_F - 128 * (MT_HALF - 1))
                    ps = mk_psum(f"ps3_{out_row0}_{mi}")
                    for k in range(4):
                        for n in range(NB):
                            nc.tensor.matmul(
                                ps[:, n * 512:(n + 1) * 512],
                                w3k[:, mi2 * 512 + k * 128: mi2 * 512 + (k + 1) * 128],
                                d2[k][:, n * 512:(n + 1) * 512],
                                start=(k == 0),
                                stop=(k == 3),
                            )
                    osl = o[:mw, mi2 * B:(mi2 + 1) * B]
                    bias_ap = bias_sb[:mw, mi:mi + 1]
                    if softplus:
                        sc = rp.tile(
                            [128, B], F32, name=f"sc_{out_row0}_{mi}", tag="sc"
                        )
                        nc.scalar.activation(sc[:mw, :], ps[:mw, :], EXP, bias=bias_ap)
                        nc.vector.tensor_scalar(
                            sc[:mw, :], sc[:mw, :], SP_C, SP_C, op0=MULT, op1=ADD
                        )
                        nc.scalar.activation(osl, sc[:mw, :], LN)
                    elif split_store and mi2 == 0:
                        # kernel tail: ACT is done with vars by now — run the
                        # two last pairs' means epilogues on ACT and DVE in
                        # parallel instead of serially on DVE
                        nc.scalar.activation(osl, ps[:mw, :], IDENT, bias=bias_ap)
                    else:
                        # means epilogue entirely on DVE (ACT is vars-bound)
                        nc.vector.tensor_scalar_add(osl, ps[:mw, :], bias_ap)
                r0 = out_row0 + 2 * j * 128
                if j < MT_HALF // 2 - 1:
                    if split_store:
                        # per-mi stores so the first half streams out while
                        # the second half's epilogue still runs
                        nc.sync.dma_start(out=yt[r0:r0 + 128, :], in_=o[:, :B])
                        nc.sync.dma_start(out=yt[r0 + 128:r0 + 256, :], in_=o[:, B:])
                    else:
                        # both mi full: one DMA writes 256 DRAM rows
                        nc.sync.dma_start(
                            out=yt[r0:r0 + 256, :].rearrange("(t p) b -> p t b", p=128),
                            in_=o.rearrange("p (t b) -> p t b", t=2),
                        )
                else:
                    nc.sync.dma_start(out=yt[r0:r0 + 128, :], in_=o[:, :B])
                    tail = IN_F - 128 * (MT_HALF - 1)
                    nc.sync.dma_start(
                        out=yt[r0 + 128:r0 + 128 + tail, :],
                        in_=o[:tail, B:],
                    )

            NJ = MT_HALF // 2
            slots = [("v", 0), ("v", 1)]
            for k in range(NJ - 2):
                slots.append(("m", k))
                slots.append(("v", k + 2))
            slots.append(("m", NJ - 2))
            slots.append(("m", NJ - 1))
            for si, (kind, j) in enumerate(slots):
                last2 = si >= len(slots) - 2
                if kind == "v":
                    dec3_pair(w3v, b3v_sb, IN_F, True, j, last2)
                else:
                    dec3_pair(w3m, b3m_sb, 0, False, j, last2)

    split_sync_waits(nc)
    return nc


# ---------------------------------------------------------------------------
# Host glue
# ---------------------------------------------------------------------------
_NC_CACHE = []


def _get_nc():
    if not _NC_CACHE:
        _NC_CACHE.append(build_bass())
    return _NC_CACHE[0]


def _bias_cols(b, ntiles):
    """[D] -> [128, ntiles]; col j = b[j*128:(j+1)*128], zero-padded."""
    out = np.zeros((128, ntiles), np.float32)
    b = np.asarray(b, np.float32)
    for j in range(ntiles):
        seg = b[j * 128:min((j + 1) * 128, b.shape[0])]
        out[: seg.shape[0], j] = seg
    return out


def _prep_shared(inputs):
    f32 = lambda a: np.ascontiguousarray(np.asarray(a), dtype=np.float32)
    bf16 = ml_dtypes.bfloat16
    w1 = f32(inputs["enc_W1"])
    w2 = f32(inputs["enc_W2"])
    tw = f32(inputs["T_W"])
    dw1 = f32(inputs["dec_W1"])
    dw2 = f32(inputs["dec_W2"])
    w3 = f32(inputs["dec_W3"])

    # w1 zero-padded to [5120, 512], packed 4 k-tiles per step:
    # w1p[j, p, u*512 + c] = W1[(4j+u)*128 + p, c]
    w1z = np.zeros((IN_FP, N0), np.float32)
    w1z[:IN_F] = w1
    w1p = np.ascontiguousarray(
        w1z.reshape(KP4, 4, 128, N0).transpose(0, 2, 1, 3).reshape(KP4, 128, 4 * N0)
    ).astype(bf16)

    # T_W packed: twp[t, p, k*256 + c] = T_W[t, k*128 + p, c]
    twp = np.ascontiguousarray(
        tw.reshape(T, 2, 128, N2).transpose(0, 2, 1, 3).reshape(T, 128, 2 * N2)
    ).astype(bf16)

    # dec_W3 halves packed in mi-pairs:
    # w3p[j, p, mi2*512 + k*128 + c] = W3[k*128 + p, (2j+mi2)*128 + c]
    def tile_w3(cols):
        out = np.zeros((MT_HALF // 2, 128, 1024), np.float32)
        for k in range(4):
            blk = cols[k * 128:(k + 1) * 128, :]          # [128, <=5120]
            cw = blk.shape[1]
            padded = np.zeros((128, MT_HALF * 128), np.float32)
            padded[:, :cw] = blk
            per_mi = padded.reshape(128, MT_HALF, 128).transpose(1, 0, 2)
            for mi2 in range(2):
                out[:, :, mi2 * 512 + k * 128: mi2 * 512 + (k + 1) * 128] = (
                    per_mi[mi2::2]
                )
        return np.ascontiguousarray(out).astype(bf16)

    w3m = tile_w3(w3[:, :IN_F])
    w3v = tile_w3(w3[:, IN_F:])

    # gate over the FULL batch (apply_t uses full-batch counts)
    treat = np.asarray(inputs["treatment"])
    tvals = np.arange(1, T + 1)
    mask = (treat[:, None, :] == tvals[None, :, None]).any(-1)  # [B, T]
    apply_t = mask.sum(0) > 1
    gate = (mask & apply_t[None, :]).astype(np.float32)         # [B, T]
    gm_full = np.ascontiguousarray((gate.T - 1.0) * 1e30)       # [T, B]

    shared = {
        "w1p": w1p,
        "w2": w2.astype(bf16),
        "twp": twp,
        "dw1": dw1.astype(bf16),
        "dw2": dw2.astype(bf16),
        "w3m": w3m,
        "w3v": w3v,
        "b1c": _bias_cols(inputs["enc_b1"], 4),
        "b2c": _bias_cols(inputs["enc_b2"], 2),
        "tbc": np.ascontiguousarray(
            np.asarray(inputs["T_b"], dtype=np.float32)
            .reshape(T, 2, 128)
            .transpose(2, 0, 1)
            .reshape(128, T * 2)
        ),
        "db1c": _bias_cols(inputs["dec_b1"], 4),
        "db2c": _bias_cols(inputs["dec_b2"], 4),
        "b3mc": _bias_cols(np.asarray(inputs["dec_b3"])[:IN_F], MT_HALF),
        "b3vc": _bias_cols(np.asarray(inputs["dec_b3"])[IN_F:], MT_HALF),
    }
    x = f32(inputs["input"])
    in_maps = []
    for c in range(N_CORES):
        m = dict(shared)
        # xT zero-padded to [5120, B], packed 4 k-tiles per step:
        # xp[j, p, u*B + c] = xT[(4j+u)*128 + p, c]
        xt = np.zeros((IN_FP, B), np.float32)
        xt[:IN_F] = x[c * B:(c + 1) * B, :].T
        m["xp"] = np.ascontiguousarray(
            xt.reshape(KP4, 4, 128, B).transpose(0, 2, 1, 3).reshape(KP4, 128, 4 * B)
        ).astype(bf16)
        gmr = np.zeros((128, B), np.float32)
        for i in range(4):
            gmr[32 * i:32 * i + T] = gm_full[:, c * B:(c + 1) * B]
        m["gmrep"] = np.ascontiguousarray(gmr).astype(bf16)
        in_maps.append(m)
    ohs = np.zeros((128, T * 128), np.float32)
    for i in range(4):
        for t in range(T):
            ohs[32 * i + t, t * 128:(t + 1) * 128] = 1.0
    ohs = np.ascontiguousarray(ohs).astype(bf16)
    for m in in_maps:
        m["ohsel"] = ohs
    return in_maps


def kernel(**inputs) -> np.ndarray:
    nc = _get_nc()
    in_maps = _prep_shared(inputs)
    res = run_bass_kernel_spmd(nc, in_maps, core_ids=list(range(N_CORES)))
    out = np.empty((BATCH, 2 * IN_F), np.float32)
    for c in range(N_CORES):
        out[c * B:(c + 1) * B, :] = res.results[c]["yt"].T.astype(np.float32)
    return out
